# revision 4
# baseline (speedup 1.0000x reference)
"""GQA sigmoid-attention (causal zero-fill) Trainium2 Bass kernel.

Problem: nn_Attention (B=2, S=2048, D=2048, 16 q-heads / 4 kv-heads, head_dim=128)
    xq = query @ Wq.T ; xk = key @ Wk.T ; xv = value @ Wv.T   (GQA repeat 4x)
    scores = sigmoid((xq xk^T) / sqrt(128)); causal zero-fill AFTER sigmoid
    out = (scores @ xv) @ Wo.T

Sharding (8 NeuronCores): core = (b, g) with b in {0,1} batches and g in {0..3}
kv-groups. Each core owns 4 query heads + their 1 kv head for one batch and
computes a partial output [S, D] through its Wo row-slice; the host sums the 4
partials per batch (the "all-reduce" of the row-sharded Wo).

Device dataflow is transpose-free: the host feeds pre-transposed views
(qT/kT/vT = x[b].T, WqT/WkT/WvT column slices of W.T, WoT row slice of Wo.T)
so every matmul has its contraction dim on SBUF partitions with contiguous
DMAs. All matmuls run in float32r (TF32-like, 1 cycle/row at free dim >= 256,
4x faster than fp32; rel err ~1.5e-4). Sigmoid runs on ScalarE with the
1/sqrt(128) scale folded in; causal masking is a gpsimd affine_select
zero-fill on the diagonal probability tiles; fully-masked tiles are skipped.
"""

import math

import numpy as np

import concourse.bacc as bacc
import concourse.mybir as mybir
import concourse.tile as tile
from concourse.bass_utils import run_bass_kernel_spmd
from concourse.masks import make_identity

B = 2
S = 2048
D = 2048
NH = 16
NKV = 4
C = 128          # head dim
HPG = NH // NKV  # 4 query heads per kv group (= per core)
F = HPG * C      # 512 query-proj dims per core
SCALE = 1.0 / math.sqrt(C)
P = 128
DT = D // P      # 16 contraction chunks
J4 = S // 512    # 4 query tiles of 512
ST = S // P      # 16 s-chunks

f32 = mybir.dt.float32
f32r = mybir.dt.float32r

_CACHE: dict = {}


def _build_module():
    nc = bacc.Bacc("TRN2", target_bir_lowering=False, debug=False, num_devices=8)

    qT = nc.dram_tensor("qT", [D, S], f32, kind="ExternalInput")
    kT = nc.dram_tensor("kT", [D, S], f32, kind="ExternalInput")
    vT = nc.dram_tensor("vT", [D, S], f32, kind="ExternalInput")
    wqT = nc.dram_tensor("wqT", [D, F], f32, kind="ExternalInput")
    wkT = nc.dram_tensor("wkT", [D, C], f32, kind="ExternalInput")
    wvT = nc.dram_tensor("wvT", [D, C], f32, kind="ExternalInput")
    woT = nc.dram_tensor("woT", [F, D], f32, kind="ExternalInput")
    out = nc.dram_tensor("out", [S, D], f32, kind="ExternalOutput")

    qT_r = qT.rearrange("(dt p) s -> p dt s", p=P)
    kT_r = kT.rearrange("(dt p) s -> p dt s", p=P)
    vT_r = vT.rearrange("(dt p) s -> p dt s", p=P)

    with tile.TileContext(nc) as tc:
        with (
            tc.tile_pool(name="consts", bufs=1) as consts,
            tc.tile_pool(name="weights", bufs=1) as wpool,
            tc.tile_pool(name="persist", bufs=1) as xpool,
            tc.tile_pool(name="attn_sb", bufs=2) as apool,
            tc.tile_pool(name="stream", bufs=3) as stream,
            tc.tile_pool(name="vtr", bufs=2) as vtr,
            tc.tile_pool(name="probs", bufs=6) as probs,
            tc.tile_pool(name="oevac", bufs=3) as oevac,
            tc.tile_pool(name="ps_proj", bufs=4, space="PSUM") as ps_proj,
            tc.tile_pool(name="ps_sc", bufs=2, space="PSUM") as ps_sc,
            tc.tile_pool(name="ps_attn", bufs=1, space="PSUM") as ps_attn,
            tc.tile_pool(name="ps_out", bufs=1, space="PSUM") as ps_out,
        ):
            ident = consts.tile([P, P], f32)
            make_identity(nc, ident)

            # Weights resident in SBUF as f32r (cast during DMA on gpsimd).
            wq_sb = wpool.tile([P, DT, F], f32r, tag="wq")
            wk_sb = wpool.tile([P, DT, C], f32r, tag="wk")
            wv_sb = wpool.tile([P, DT, C], f32r, tag="wv")
            wo_sb = wpool.tile([P, HPG, D], f32r, tag="wo")
            nc.gpsimd.dma_start(wq_sb[:], wqT.rearrange("(dt p) f -> p dt f", p=P))
            nc.gpsimd.dma_start(wk_sb[:], wkT.rearrange("(dt p) c -> p dt c", p=P))
            nc.gpsimd.dma_start(wv_sb[:], wvT.rearrange("(dt p) c -> p dt c", p=P))
            nc.gpsimd.dma_start(wo_sb[:], woT.rearrange("(g p) d -> p g d", p=P))

            # Persistent projected tensors (f32r).
            xqT = xpool.tile([P, HPG, S], f32r, tag="xqT")   # [c, h, q]
            xkT = xpool.tile([P, S], f32r, tag="xkT")        # [c, k]
            xv = xpool.tile([P, ST, C], f32r, tag="xv")      # [k%128, kchunk, c]

            for j in range(J4):
                sl = slice(j * 512, (j + 1) * 512)

                # ---- Q projection for this 512-col block: 4 psum banks, dt inner
                ps_q = [ps_proj.tile([P, 512], f32, tag="proj", name=f"psq{h_}") for h_ in range(HPG)]
                for dt in range(DT):
                    qc = stream.tile([P, 512], f32r, tag="qc")
                    nc.gpsimd.dma_start(qc[:], qT_r[:, dt, sl])
                    for h in range(HPG):
                        nc.tensor.matmul(
                            ps_q[h][:], wq_sb[:, dt, h * P:(h + 1) * P], qc[:],
                            start=(dt == 0), stop=(dt == DT - 1),
                        )
                for h in range(HPG):
                    nc.vector.tensor_copy(xqT[:, h, sl], ps_q[h][:])

                # ---- K + V projections: 2 psum banks, dt inner
                ps_k = ps_proj.tile([P, 512], f32, tag="proj")
                ps_v = ps_proj.tile([P, 512], f32, tag="proj")
                for dt in range(DT):
                    kc = stream.tile([P, 512], f32r, tag="kc")
                    vc = stream.tile([P, 512], f32r, tag="vc")
                    nc.gpsimd.dma_start(kc[:], kT_r[:, dt, sl])
                    nc.gpsimd.dma_start(vc[:], vT_r[:, dt, sl])
                    nc.tensor.matmul(ps_k[:], wk_sb[:, dt, :], kc[:],
                                     start=(dt == 0), stop=(dt == DT - 1))
                    nc.tensor.matmul(ps_v[:], wv_sb[:, dt, :], vc[:],
                                     start=(dt == 0), stop=(dt == DT - 1))
                nc.vector.tensor_copy(xkT[:, sl], ps_k[:])

                # V arrives transposed [c, s]; PE-transpose 128x128 tiles into
                # k-major xv (needed as AV lhsT).
                xvT_sb = vtr.tile([P, 512], f32, tag="xvT")
                nc.vector.tensor_copy(xvT_sb[:], ps_v[:])
                for sc in range(4):
                    pst = ps_sc.tile([P, P], f32, tag="sc")
                    nc.tensor.transpose(pst[:], xvT_sb[:, sc * P:(sc + 1) * P], ident[:])
                    nc.vector.tensor_copy(xv[:, j * 4 + sc, :], pst[:])

                # ---- attention for query tile j (all heads); causal skip k>q
                nk = 4 * (j + 1)
                at_block = apool.tile([P, HPG, 512], f32r, tag="attnT")
                for h in range(HPG):
                    ps_at = ps_attn.tile([P, 512], f32, tag="at")
                    for kc_i in range(nk):
                        ps_s = ps_sc.tile([P, 512], f32, tag="sc")
                        nc.tensor.matmul(
                            ps_s[:], xkT[:, kc_i * P:(kc_i + 1) * P], xqT[:, h, sl],
                            start=True, stop=True,
                        )
                        pr = probs.tile([P, 512], f32r, tag="pr")
                        nc.scalar.activation(
                            pr[:], ps_s[:], mybir.ActivationFunctionType.Sigmoid,
                            scale=float(SCALE),
                        )
                        r = kc_i - 4 * j
                        if r >= 0:
                            # diagonal tile: zero where k > q, i.e. keep where
                            # (q_in_tile - k_in_chunk - 128 r) >= 0
                            nc.gpsimd.affine_select(
                                out=pr[:], in_=pr[:],
                                compare_op=mybir.AluOpType.is_ge,
                                fill=0.0, base=-P * r, channel_multiplier=-1,
                                pattern=[[1, 512]],
                            )
                        nc.tensor.matmul(ps_at[:], xv[:, kc_i, :], pr[:],
                                         start=(kc_i == 0), stop=(kc_i == nk - 1))
                    nc.vector.tensor_copy(at_block[:, h, :], ps_at[:])

                # ---- output projection for this block of 512 rows
                for s16 in range(4):
                    row0 = (j * 4 + s16) * P
                    for n4 in range(4):
                        ps_o = ps_out.tile([P, 512], f32, tag="o")
                        for h in range(HPG):
                            nc.tensor.matmul(
                                ps_o[:],
                                at_block[:, h, s16 * P:(s16 + 1) * P],
                                wo_sb[:, h, n4 * 512:(n4 + 1) * 512],
                                start=(h == 0), stop=(h == HPG - 1),
                            )
                        ot = oevac.tile([P, 512], f32, tag="ot")
                        nc.vector.tensor_copy(ot[:], ps_o[:])
                        nc.sync.dma_start(
                            out[row0:row0 + P, n4 * 512:(n4 + 1) * 512], ot[:]
                        )
    nc.compile()
    return nc


def _get_module():
    if "nc" not in _CACHE:
        _CACHE["nc"] = _build_module()
    return _CACHE["nc"]


def make_in_maps(query, key, value, Wq, Wk, Wv, Wo):
    """Host-side sharding: per-core input dict (core = b*4 + g)."""
    query = np.asarray(query, dtype=np.float32)
    key = np.asarray(key, dtype=np.float32)
    value = np.asarray(value, dtype=np.float32)
    Wq = np.asarray(Wq, dtype=np.float32)
    Wk = np.asarray(Wk, dtype=np.float32)
    Wv = np.asarray(Wv, dtype=np.float32)
    Wo = np.asarray(Wo, dtype=np.float32)

    qT = [np.ascontiguousarray(query[b].T) for b in range(B)]
    kTb = [np.ascontiguousarray(key[b].T) for b in range(B)]
    vTb = [np.ascontiguousarray(value[b].T) for b in range(B)]
    WqT = np.ascontiguousarray(Wq.T)  # [D, NH*C]
    WkT = np.ascontiguousarray(Wk.T)  # [D, NKV*C]
    WvT = np.ascontiguousarray(Wv.T)
    WoT = np.ascontiguousarray(Wo.T)  # [D_in, D_out]

    in_maps = []
    for core in range(8):
        b, g = divmod(core, 4)
        in_maps.append({
            "qT": qT[b],
            "kT": kTb[b],
            "vT": vTb[b],
            "wqT": np.ascontiguousarray(WqT[:, g * F:(g + 1) * F]),
            "wkT": np.ascontiguousarray(WkT[:, g * C:(g + 1) * C]),
            "wvT": np.ascontiguousarray(WvT[:, g * C:(g + 1) * C]),
            "woT": np.ascontiguousarray(WoT[g * F:(g + 1) * F, :]),
        })
    return in_maps


def kernel(**inputs) -> np.ndarray:
    nc = _get_module()
    in_maps = make_in_maps(**inputs)
    res = run_bass_kernel_spmd(nc, in_maps, core_ids=list(range(8)))
    parts = [res.results[c]["out"] for c in range(8)]
    full = np.empty((B, S, D), dtype=np.float32)
    for b in range(B):
        full[b] = parts[b * 4] + parts[b * 4 + 1] + parts[b * 4 + 2] + parts[b * 4 + 3]
    return full


# revision 23
# speedup vs baseline: 986.7128x; 986.7128x over previous
"""GQA sigmoid-attention (causal zero-fill) Trainium2 Bass kernel.

Problem: nn_Attention (B=2, S=2048, D=2048, 16 q-heads / 4 kv-heads, head_dim=128)
    xq = query @ Wq.T ; xk = key @ Wk.T ; xv = value @ Wv.T   (GQA repeat 4x)
    scores = sigmoid((xq xk^T) / sqrt(128)); causal zero-fill AFTER sigmoid
    out = (scores @ xv) @ Wo.T

Sharding (8 NeuronCores): core = (b, g) with b in {0,1} batches and g in {0..3}
kv-groups. Each core owns 4 query heads + their 1 kv head for one batch and
computes a partial output [S, D] through its Wo row-slice; the host sums the 4
partials per batch (the "all-reduce" of the row-sharded Wo).

Device dataflow is transpose-free: the host feeds pre-transposed, pre-packed
views (qT/kT/vT = x[b].T; weights packed to the exact SBUF partition layout)
so every matmul has its contraction dim on SBUF partitions. All matmuls run in
float32r (TF32-like, 1 cycle/row at free dim >= 256, 4x faster than fp32; rel
err ~1.5e-4), with fp32->f32r rounding done inside the gpsimd cast-DMAs.
Sigmoid runs on ScalarE with the 1/sqrt(128) scale folded in; causal masking
is a DVE multiply with precomputed per-diagonal-offset mask tiles;
fully-masked tiles are skipped (halves attention FLOPs).

Software-pipelined over the 4 query tiles j (512 rows each):
  Qproj(j):  xqT_j[c,h,q] += WqT-chunk^T @ qT-chunk  (16 d-chunks, 4 PSUM banks)
  KVproj(j): xkT[c,k], xvT -> PE-transpose -> xv[k,c] (2 PSUM banks)
  B(j):      per head: scoresT[k,q] (PE) -> sigmoid (ScalarE, psum->sbuf,
             f32r) -> diagonal mask (DVE) -> attnT[c,q] accumulation (PE)
  C(j):      out[s,dout] = attnT^T @ WoT, 4-head accumulation; interleaved
             with B(j+1) head groups so C's matmuls fill B's ACT-latency gaps.
Deep qc prefetch (16 bufs) lets j+1's DMA stream entirely under B(j)/C(j-1)
compute; DMA, PE, ACT, DVE all stay busy concurrently.
"""

import math

import numpy as np

import concourse.bacc as bacc
import concourse.mybir as mybir
import concourse.tile as tile
from concourse.bass_utils import run_bass_kernel_spmd
from concourse.masks import make_identity

B = 2
S = 2048
D = 2048
NH = 16
NKV = 4
C = 128          # head dim
HPG = NH // NKV  # 4 query heads per kv group (= per core)
F = HPG * C      # 512 query-proj dims per core
SCALE = 1.0 / math.sqrt(C)
P = 128
DT = D // P      # 16 contraction chunks
J4 = S // 512    # 4 query tiles of 512
ST = S // P      # 16 s-chunks

f32 = mybir.dt.float32
f32r = mybir.dt.float32r

_CACHE: dict = {}

# timing-experiment knobs (production path leaves these alone)
_OPTS = {"phases": "ABC", "sigmoid": True, "mask": True, "b_order": "h", "c_interleave": True, "w_hwdge": False, "kv_hw": False, "q_hw": False}


def _build_module(n_iters: int = 0, internal_io: bool = False):
    """Build the per-core module. n_iters=0: straight-line kernel (production).
    n_iters>0: wrap the whole body in a For_i repeat loop (timing variant —
    per-iteration wall-clock slope measures true on-device exec time).
    internal_io=True replaces the big I/O tensors with on-device scratch so
    a timing call transfers almost nothing over the wire."""
    nc = bacc.Bacc("TRN2", target_bir_lowering=False, debug=False, num_devices=8)

    # weights arrive pre-packed to SBUF layout: [128, DT, *] / [128, HPG, D]
    if internal_io:
        dummy_in = nc.dram_tensor("dummy_in", [1, 1], f32, kind="ExternalInput")
        dummy_out = nc.dram_tensor("dummy_out", [1, 1], f32, kind="ExternalOutput")
        kw = {}
    else:
        kw = {"kind": "ExternalInput"}
    qT = nc.dram_tensor("qT", [D, S], f32, **kw)
    kT = nc.dram_tensor("kT", [D, S], f32, **kw)
    vT = nc.dram_tensor("vT", [D, S], f32, **kw)
    wqP = nc.dram_tensor("wqP", [P, DT, F], f32, **kw)
    wkP = nc.dram_tensor("wkP", [P, DT, C], f32, **kw)
    wvP = nc.dram_tensor("wvP", [P, DT, C], f32, **kw)
    woP = nc.dram_tensor("woP", [P, HPG, D], f32, **kw)
    if internal_io:
        out = nc.dram_tensor("out", [S, D], f32)
    else:
        out = nc.dram_tensor("out", [S, D], f32, kind="ExternalOutput")

    qT_r = qT.rearrange("(dt p) s -> p dt s", p=P)
    kT_r = kT.rearrange("(dt p) s -> p dt s", p=P)
    vT_r = vT.rearrange("(dt p) s -> p dt s", p=P)

    do_B = "B" in _OPTS["phases"]
    do_C = "C" in _OPTS["phases"]

    with tile.TileContext(nc) as tc:
        with (
            tc.tile_pool(name="consts", bufs=1) as consts,
            tc.tile_pool(name="weights", bufs=1) as wpool,
            tc.tile_pool(name="xkv", bufs=1) as xkv_pool,
            tc.tile_pool(name="xq", bufs=2) as xq_pool,
            tc.tile_pool(name="attn_sb", bufs=2) as apool,
            tc.tile_pool(name="qstream", bufs=6) as qstream,
            tc.tile_pool(name="kvstream", bufs=_OPTS.get("kv_bufs", 6)) as kvstream,
            tc.tile_pool(name="kvstage", bufs=4) as kvstage,
            tc.tile_pool(name="vtr", bufs=2) as vtr,
            tc.tile_pool(name="probs", bufs=8) as probs,
            tc.tile_pool(name="oevac", bufs=3) as oevac,
            tc.tile_pool(name="ps8", bufs=4, space="PSUM") as ps8,
            tc.tile_pool(name="ps_kv", bufs=2, space="PSUM") as ps_kv_pool,
            tc.tile_pool(name="ps_sc", bufs=2, space="PSUM") as ps_sc_pool,
        ):
          def emit_c(at_prev, j, s16):
              """C(j, s16): one 128-row group of the output projection.
              n4 pairs share the stationary operand per h (LDW amortized)."""
              row0 = (j * 4 + s16) * P
              for np_ in range(2):
                  pool_ = ps8 if np_ == 0 else ps_kv_pool
                  tag_ = "x" if np_ == 0 else "kv"
                  ps_o = [pool_.tile([P, 512], f32, tag=tag_, name=f"ps_o{i}")
                          for i in range(2)]
                  for h in range(HPG):
                      for i in range(2):
                          n4 = np_ * 2 + i
                          nc.tensor.matmul(
                              ps_o[i][:],
                              at_prev[:, h, s16 * P:(s16 + 1) * P],
                              wo_sb[:, h, n4 * 512:(n4 + 1) * 512],
                              start=(h == 0), stop=(h == HPG - 1))
                  for i in range(2):
                      n4 = np_ * 2 + i
                      ot = oevac.tile([P, 512], f32, tag="ot", name="ot")
                      nc.vector.tensor_copy(ot[:], ps_o[i][:])
                      nc.sync.dma_start(
                          out[row0:row0 + P, n4 * 512:(n4 + 1) * 512], ot[:])

          def body(_iv=None):
            global wo_sb
            ident = consts.tile([P, P], f32, name="ident")
            make_identity(nc, ident)
            masks = consts.tile([P, J4, 512], f32, name="masks")

            # K/V/Q weights first (small, unblock projections ASAP)
            wk_sb = wpool.tile([P, DT, C], f32r, tag="wk", name="wk_sb")
            wv_sb = wpool.tile([P, DT, C], f32r, tag="wv", name="wv_sb")
            wq_sb = wpool.tile([P, DT, F], f32r, tag="wq", name="wq_sb")
            wo_sb = wpool.tile([P, HPG, D], f32r, tag="wo", name="wo_sb")
            def load_f32r(dst, src_ap, hwdge):
                """Load fp32 DRAM into an f32r SBUF tile: gpsimd cast-DMA, or
                HWDGE raw-bits DMA + in-place DVE rounding pass."""
                if hwdge:
                    nc.sync.dma_start(dst.bitcast(f32)[:], src_ap)
                    nc.vector.tensor_copy(dst[:], dst.bitcast(f32)[:])
                else:
                    nc.gpsimd.dma_start(dst[:], src_ap)

            load_f32r(wk_sb, wkP[:], _OPTS["w_hwdge"])
            load_f32r(wv_sb, wvP[:], _OPTS["w_hwdge"])

            xkT = xkv_pool.tile([P, S], f32r, tag="xkT", name="xkT")    # [c,k]
            xv = xkv_pool.tile([P, ST, C], f32r, tag="xv", name="xv")   # [k%128,kc,c]

            at_prev = None
            for j in range(J4):
                # ---- KVproj(j): 2 PSUM banks; loads on HWDGE + DVE cast
                ps_k = ps_kv_pool.tile([P, 512], f32, tag="kv", name="ps_k")
                ps_v = ps_kv_pool.tile([P, 512], f32, tag="kv", name="ps_v")
                for dt in range(DT):
                    kc = kvstream.tile([P, 512], f32r, tag="kc", name="kc")
                    vc = kvstream.tile([P, 512], f32r, tag="vc", name="vc")
                    if _OPTS["kv_hw"]:
                        ks = kvstage.tile([P, 512], f32, tag="kvs", name="ks")
                        nc.sync.dma_start(ks[:], kT_r[:, dt, j * 512:(j + 1) * 512])
                        nc.vector.tensor_copy(kc[:], ks[:])
                        vs = kvstage.tile([P, 512], f32, tag="kvs", name="vs")
                        nc.sync.dma_start(vs[:], vT_r[:, dt, j * 512:(j + 1) * 512])
                        nc.vector.tensor_copy(vc[:], vs[:])
                    else:
                        nc.gpsimd.dma_start(kc[:], kT_r[:, dt, j * 512:(j + 1) * 512])
                        nc.gpsimd.dma_start(vc[:], vT_r[:, dt, j * 512:(j + 1) * 512])
                    st, sp = dt == 0, dt == DT - 1
                    nc.tensor.matmul(ps_k[:], wk_sb[:, dt, :], kc[:], start=st, stop=sp)
                    nc.tensor.matmul(ps_v[:], wv_sb[:, dt, :], vc[:], start=st, stop=sp)
                if j == 0:
                    # wq issued after KV(0)'s chunk DMAs: Qproj(0) needs it
                    # ~40us in; issuing it first would delay KV(0) by ~15us
                    load_f32r(wq_sb, wqP[:], _OPTS["w_hwdge"])
                nc.vector.tensor_copy(xkT[:, j * 512:(j + 1) * 512], ps_k[:])
                xvT_sb = vtr.tile([P, 512], f32, tag="xvT", name="xvT_sb")
                nc.vector.tensor_copy(xvT_sb[:], ps_v[:])
                for sc in range(4):
                    pst = ps_kv_pool.tile([P, P], f32, tag="kv", name="pst")
                    nc.tensor.transpose(pst[:], xvT_sb[:, sc * P:(sc + 1) * P],
                                        ident[:])
                    nc.vector.tensor_copy(xv[:, j * 4 + sc, :], pst[:])

                if j == 0:
                    # wo arrives during B(0)/C(0) — needed first at C(0)
                    load_f32r(wo_sb, woP[:], _OPTS["w_hwdge"])
                    # causal masks for the diagonal 128x512 tiles: keep (k <= q)
                    # i.e. mask_r[i, jq] = 1 iff jq - i - 128 r >= 0
                    # (emitted here so gpsimd's queue issues the phase-A DMAs
                    # first; masks are first needed at B(0))
                    nc.gpsimd.memset(masks[:], 1.0)
                    for r in range(J4):
                        nc.gpsimd.affine_select(
                            out=masks[:, r, :], in_=masks[:, r, :],
                            compare_op=mybir.AluOpType.is_ge,
                            fill=0.0, base=-P * r, channel_multiplier=-1,
                            pattern=[[1, 512]])

                # ---- Qproj(j): 4 PSUM banks, d-chunk inner
                xqT_j = xq_pool.tile([P, HPG, 512], f32r, tag="xqT", name="xqT_j")
                ps_q = [ps8.tile([P, 512], f32, tag="x", name=f"psq{h_}")
                        for h_ in range(HPG)]
                for dt in range(DT):
                    qc = qstream.tile([P, 512], f32r, tag="qc", name="qc")
                    load_f32r(qc, qT_r[:, dt, j * 512:(j + 1) * 512], _OPTS["q_hw"])
                    for h in range(HPG):
                        nc.tensor.matmul(ps_q[h][:], wq_sb[:, dt, h * P:(h + 1) * P],
                                         qc[:], start=(dt == 0), stop=(dt == DT - 1))
                for h in range(HPG):
                    nc.vector.tensor_copy(xqT_j[:, h, :], ps_q[h][:])

                if not do_B:
                    continue
                nk = 4 * (j + 1)
                at_block = apool.tile([P, HPG, 512], f32r, tag="attnT",
                                      name="at_block")

                def score_prob(kc_i, h):
                    # diagonal tiles (r >= 0): columns < 128 r are fully
                    # masked -> compute only cols >= c0 (c0=0 for r in {0,3}:
                    # r=3 would leave N=128 < 256, the f32r slow-mode cliff)
                    r = kc_i - 4 * j
                    c0 = 128 * r if r in (1, 2) else 0
                    w = 512 - c0
                    ps_s = ps_sc_pool.tile([P, 512], f32, tag="sc", name="ps_s")
                    nc.tensor.matmul(
                        ps_s[:, c0:], xkT[:, kc_i * P:(kc_i + 1) * P],
                        xqT_j[:, h, c0:], start=True, stop=True)
                    pr = probs.tile([P, 512], f32r, tag="pr", name="pr")
                    if _OPTS["sigmoid"]:
                        nc.scalar.activation(
                            pr[:, c0:], ps_s[:, c0:],
                            mybir.ActivationFunctionType.Sigmoid,
                            scale=float(SCALE))
                    else:
                        nc.vector.tensor_copy(pr[:, c0:], ps_s[:, c0:])
                    if r >= 0 and _OPTS["mask"]:
                        nc.vector.tensor_mul(
                            out=pr[:, c0:], in0=pr[:, c0:], in1=masks[:, r, c0:])
                    return pr, c0

                if _OPTS["b_order"] == "kc":
                    # kc-outer / h-inner: scores share xkT[kc], AV share xv[kc]
                    ps_at = [ps8.tile([P, 512], f32, tag="x", name=f"ps_at{h_}")
                             for h_ in range(HPG)]
                    for kc_i in range(nk):
                        prs = [score_prob(kc_i, h) for h in range(HPG)]
                        for h in range(HPG):
                            pr, c0 = prs[h]
                            nc.tensor.matmul(ps_at[h][:, c0:], xv[:, kc_i, :],
                                             pr[:, c0:], start=(kc_i == 0),
                                             stop=(kc_i == nk - 1))
                    for h in range(HPG):
                        nc.vector.tensor_copy(at_block[:, h, :], ps_at[h][:])
                    if do_C:
                        for s16 in range(4):
                            emit_c(at_block, j, s16)
                else:
                    # h-outer: one attn accumulator at a time
                    for h in range(HPG):
                        ps_at = ps8.tile([P, 512], f32, tag="x", name="ps_at")
                        for kc_i in range(nk):
                            pr, c0 = score_prob(kc_i, h)
                            nc.tensor.matmul(ps_at[:, c0:], xv[:, kc_i, :],
                                             pr[:, c0:], start=(kc_i == 0),
                                             stop=(kc_i == nk - 1))
                        nc.vector.tensor_copy(at_block[:, h, :], ps_at[:])
                        if do_C and _OPTS["c_interleave"] and at_prev is not None:
                            emit_c(at_prev, j - 1, h)
                    if do_C and not _OPTS["c_interleave"]:
                        for s16 in range(4):
                            emit_c(at_block, j, s16)
                at_prev = at_block

            if (do_B and do_C and _OPTS["b_order"] == "h"
                    and _OPTS["c_interleave"]):
                for s16 in range(4):
                    emit_c(at_prev, J4 - 1, s16)

          if internal_io:
              dt_ = consts.tile([1, 1], f32, name="dt_")
              nc.sync.dma_start(dt_[:], dummy_in[:])
              nc.sync.dma_start(dummy_out[:], dt_[:])
          if n_iters:
              import os as _os
              _kw = {}
              if _os.environ.get("LOOP_HINTS") == "1":
                  _kw = dict(hint_engines=(mybir.EngineType.PE,
                                           mybir.EngineType.Activation,
                                           mybir.EngineType.DVE,
                                           mybir.EngineType.Pool,
                                           mybir.EngineType.SP))
              if _os.environ.get("LOOP_STAGGER") == "1":
                  _kw["staggered_reset"] = True
              with tc.For_i(0, n_iters, 1, **_kw):
                  body()
          else:
              body()
    nc.compile()
    return nc


def _get_module():
    if "nc" not in _CACHE:
        _CACHE["nc"] = _build_module()
    return _CACHE["nc"]


def _pack_w(wT: np.ndarray, free: int) -> np.ndarray:
    """[D, free] weight (already W.T slice) -> SBUF-layout [128, DT, free]."""
    return np.ascontiguousarray(wT.reshape(DT, P, free).transpose(1, 0, 2))


def make_in_maps(query, key, value, Wq, Wk, Wv, Wo):
    """Host-side sharding: per-core input dict (core = b*4 + g)."""
    query = np.asarray(query, dtype=np.float32)
    key = np.asarray(key, dtype=np.float32)
    value = np.asarray(value, dtype=np.float32)
    Wq = np.asarray(Wq, dtype=np.float32)
    Wk = np.asarray(Wk, dtype=np.float32)
    Wv = np.asarray(Wv, dtype=np.float32)
    Wo = np.asarray(Wo, dtype=np.float32)

    qT = [np.ascontiguousarray(query[b].T) for b in range(B)]
    kTb = [np.ascontiguousarray(key[b].T) for b in range(B)]
    vTb = [np.ascontiguousarray(value[b].T) for b in range(B)]
    WqT = Wq.T  # [D, NH*C]
    WkT = Wk.T  # [D, NKV*C]
    WvT = Wv.T
    WoT = Wo.T  # [D_in, D_out]

    in_maps = []
    for core in range(8):
        b, g = divmod(core, 4)
        woT_g = WoT[g * F:(g + 1) * F, :]  # [F, D]
        in_maps.append({
            "qT": qT[b],
            "kT": kTb[b],
            "vT": vTb[b],
            "wqP": _pack_w(WqT[:, g * F:(g + 1) * F], F),
            "wkP": _pack_w(WkT[:, g * C:(g + 1) * C], C),
            "wvP": _pack_w(WvT[:, g * C:(g + 1) * C], C),
            # [F, D] -> [128, HPG, D] (partition = c within head chunk)
            "woP": np.ascontiguousarray(
                woT_g.reshape(HPG, P, D).transpose(1, 0, 2)),
        })
    return in_maps


def kernel(**inputs) -> np.ndarray:
    nc = _get_module()
    in_maps = make_in_maps(**inputs)
    res = run_bass_kernel_spmd(nc, in_maps, core_ids=list(range(8)))
    parts = [res.results[c]["out"] for c in range(8)]
    full = np.empty((B, S, D), dtype=np.float32)
    for b in range(B):
        full[b] = parts[b * 4] + parts[b * 4 + 1] + parts[b * 4 + 2] + parts[b * 4 + 3]
    return full
